# revision 7
# baseline (speedup 1.0000x reference)
"""Enformer dot-product self-attention, 8 TRN2 cores, one head per core.

Fully transposed pipeline (logits computed as logits^T [j-part, i-free]):
  - Band rel-shift: T[i,c'] tiles -> DRAM G (pitch Q), read back through
    the DMA XBAR transpose with a diagonal [[Q-1,wdt],[1,128]] pattern,
    landing directly in [j, i] layout. This removes all 256 PE transpose
    matmuls + PSUM evacuations of the v1 kernel.
  - A-phase front-loaded; input loads, G writes AND the 16 XBAR reads all
    share the nc.sync HWDGE ring (FIFO) — concurrent XBAR transposes on
    two rings corrupt data, and same-ring writes avoid cross-ring
    completion chains that otherwise pace the C sweep at ~7.5us/tile.
  - C-phase uses [128,1024] two-bank PSUM tiles: 2 f32r qk matmuls +
    DVE band add, one wide exp per half (amortizes ACT overhead).
  - attn@v: lhsT = [v | ones] so softmax denominators ride along in
    PSUM row 64; final PE transpose + reciprocal scale.
  - Measured: 118.4us vs 137.6us for the v1 (P-transpose-on-PE) kernel.
"""

import numpy as np
import ml_dtypes

import concourse.bass as bass
import concourse.bacc as bacc
import concourse.mybir as mybir
import concourse.tile as tile
from concourse.bass_utils import run_bass_kernel_spmd
from concourse.masks import make_identity

S = 2048
D = 64
NB = 64
H = 8
HALF = NB // 2
BAND = 1024
Q = S + 128      # G row pitch (2049 band cols + 127 zero pad)
NT = S // 128
F32 = mybir.dt.float32
F32R = mybir.dt.float32r
BF16 = mybir.dt.bfloat16

_NC_CACHE = {}


def _basis_feature_matrix():
    pow_rate = np.float32(np.exp(np.log((S + 1) / 2) / HALF))
    widths = np.power(pow_rate, np.arange(1, HALF + 1, dtype=np.float32),
                      dtype=np.float32)
    d = (np.float32(BAND) - np.arange(Q, dtype=np.float32))[:, None]
    unsigned = (np.abs(d) <= widths[None, :]).astype(np.float32)
    signed = np.sign(d) * unsigned
    return np.concatenate([unsigned, signed], axis=-1)  # [Q, 64]


def _build_nc():
    if "nc" in _NC_CACHE:
        return _NC_CACHE["nc"]

    nc = bacc.Bacc("TRN2", target_bir_lowering=False, debug=False,
                   num_devices=H)
    d_qf = nc.dram_tensor("qaug_f", [65, S], F32R, kind="ExternalInput")
    d_qb = nc.dram_tensor("qaug_b", [65, S], BF16, kind="ExternalInput")
    d_k = nc.dram_tensor("kaug", [65, S], F32R, kind="ExternalInput")
    d_w2r = nc.dram_tensor("w2r", [65, Q], BF16, kind="ExternalInput")
    d_v = nc.dram_tensor("vaug", [S, 65], BF16, kind="ExternalInput")
    d_out = nc.dram_tensor("out", [S, D], F32, kind="ExternalOutput")
    d_G = nc.dram_tensor("gband", [S * Q], BF16, kind="Internal")

    with tile.TileContext(nc) as tc:
        with tc.tile_pool(name="pers", bufs=1) as pers:
            # sync ring: input loads first (FIFO before any XBAR use).
            sb_qb = pers.tile([65, S], BF16)
            nc.sync.dma_start(out=sb_qb[:, 0:128], in_=d_qb[:, 0:128])
            sb_w2r = pers.tile([65, Q], BF16)
            for c in (1, 2, 3, 0):
                lo, hi = c * 544, min(Q, (c + 1) * 544)
                nc.sync.dma_start(out=sb_w2r[:, lo:hi], in_=d_w2r[:, lo:hi])
            nc.sync.dma_start(out=sb_qb[:, 128:S], in_=d_qb[:, 128:S])
            sb_qf = pers.tile([65, S], F32R)
            nc.sync.dma_start(out=sb_qf[:], in_=d_qf[:])
            sb_k = pers.tile([65, S], F32R)
            nc.sync.dma_start(out=sb_k[:], in_=d_k[:])
            sb_v = pers.tile([128, NT, 65], BF16)
            rdv = bass.AP(tensor=d_v, offset=0,
                          ap=[[65, 128], [128 * 65, NT], [1, 65]])
            nc.sync.dma_start(out=sb_v[:], in_=rdv)
            sb_id = pers.tile([128, 128], F32)
            sb_PT = pers.tile([128, NT, S], BF16)   # P^T, [j-part, jb, i]

            def phase_A(t):
                i0 = t * 128
                jlo = max(0, i0 - BAND)
                jhi = min(S, i0 + 128 + BAND)
                clo = max(0, (jlo - i0 + BAND) - 127)
                chi = min(2049, (jhi - 1) - i0 + BAND + 1)
                gt = gsb.tile([128, Q], BF16)
                nc.gpsimd.memset(gt[:, chi:Q], 0.0)
                cuts = list(range(clo, chi, 1024)) + [chi]
                for ci in range(len(cuts) - 1):
                    lo, hi = cuts[ci], cuts[ci + 1]
                    pg = psC.tile([128, 1024], F32, tag="pq")
                    nsub = (hi - lo + 511) // 512
                    for si in range(nsub):
                        slo = lo + si * 512
                        shi = min(hi, slo + 512)
                        nc.tensor.matmul(
                            pg[:, slo - lo:shi - lo],
                            lhsT=sb_qb[:, i0:i0 + 128],
                            rhs=sb_w2r[:, slo:shi],
                            start=True, stop=True)
                    if (t + ci) % 2 == 0:
                        nc.scalar.copy(out=gt[:, lo:hi], in_=pg[:, 0:hi - lo])
                    else:
                        nc.vector.tensor_copy(gt[:, lo:hi], pg[:, 0:hi - lo])
                wr = bass.AP(tensor=d_G, offset=i0 * Q + clo,
                             ap=[[Q, 128], [1, Q - clo]])
                nc.sync.dma_start(out=wr, in_=gt[:, clo:Q])

            def phase_Cread(jb):
                j0 = jb * 128
                ilo = max(0, j0 - BAND)
                ihi = min(S, j0 + 128 + BAND)
                wdt = ihi - ilo
                bt = bsb.tile([128, S], BF16, tag="bt")
                rd = bass.AP(tensor=d_G, offset=ilo * (Q - 1) + j0 + BAND,
                             ap=[[Q - 1, wdt], [1, 128]])
                nc.sync.dma_start(out=bt[:, 0:wdt], in_=rd, transpose=True)
                return bt

            def phase_C(jb, bt):
                j0 = jb * 128
                ilo = max(0, j0 - BAND)
                ihi = min(S, j0 + 128 + BAND)
                for hf in range(2):
                    h0 = hf * 1024
                    pq = psC.tile([128, 1024], F32, tag="pq")
                    alo = max(ilo, h0)
                    ahi = min(ihi, h0 + 1024)
                    for c in range(2):
                        nc.tensor.matmul(
                            pq[:, c * 512:(c + 1) * 512],
                            lhsT=sb_k[:, j0:j0 + 128],
                            rhs=sb_qf[:, h0 + c * 512:h0 + (c + 1) * 512],
                            start=True, stop=True)
                    if alo < ahi:
                        nc.vector.tensor_add(
                            pq[:, alo - h0:ahi - h0],
                            pq[:, alo - h0:ahi - h0],
                            bt[:, alo - ilo:ahi - ilo])
                    nc.scalar.activation(
                        out=sb_PT[:, jb, h0:h0 + 1024], in_=pq[:],
                        func=mybir.ActivationFunctionType.Exp)

            def phase_AV(c):
                cs = c * 512
                po = psV.tile([65, 512], F32, tag="po")
                for jb in range(NT):
                    nc.tensor.matmul(
                        po[:],
                        lhsT=sb_v[:, jb, :],
                        rhs=sb_PT[:, jb, cs:cs + 512],
                        start=(jb == 0), stop=(jb == NT - 1))
                o = osb.tile([65, 512], F32, tag="oT")
                nc.scalar.copy(out=o[:], in_=po[:])
                return o

            def phase_F(c, o, ot):
                for s in range(4):
                    pf = psV.tile([128, 65], F32, tag="pf")
                    nc.tensor.transpose(pf[:, 0:65],
                                        o[:, s * 128:(s + 1) * 128],
                                        sb_id[0:65, 0:65])
                    rc = fsb.tile([128, 1], F32, tag="rc")
                    nc.vector.reciprocal(rc[:], pf[:, 64:65])
                    nc.vector.tensor_scalar_mul(ot[:, s, :], pf[:, 0:D],
                                                rc[:])
                wr = bass.AP(tensor=d_out, offset=c * 512 * D,
                             ap=[[D, 128], [128 * D, 4], [1, D]])
                nc.gpsimd.dma_start(out=wr, in_=ot[:])

            with tc.tile_pool(name="gsb", bufs=4) as gsb, \
                 tc.tile_pool(name="bsb", bufs=6) as bsb, \
                 tc.tile_pool(name="psC", bufs=4, space="PSUM") as psC:
                make_identity(nc, sb_id[:])
                for t in range(9):
                    phase_A(t)
                bts = {0: phase_Cread(0)}
                for jb in range(NT):
                    if jb + 9 < NT:
                        phase_A(jb + 9)
                    if jb + 1 < NT:
                        bts[jb + 1] = phase_Cread(jb + 1)
                    phase_C(jb, bts.pop(jb))

            with tc.tile_pool(name="osb", bufs=2) as osb, \
                 tc.tile_pool(name="fsb", bufs=2) as fsb, \
                 tc.tile_pool(name="psV", bufs=2, space="PSUM") as psV:
                os_ = [phase_AV(0), phase_AV(1)]
                for c in range(4):
                    if c + 2 < 4:
                        os_.append(phase_AV(c + 2))
                    ot = fsb.tile([128, 4, D], F32, tag="ot")
                    phase_F(c, os_[c], ot)

    nc.finalize()
    _NC_CACHE["nc"] = nc
    return nc


def _host_prep(query, key, value, u, v, w):
    q = np.asarray(query, np.float32)[0]
    k = np.asarray(key, np.float32)[0]
    val = np.asarray(value, np.float32)[0]
    u = np.asarray(u, np.float32)
    v = np.asarray(v, np.float32)
    w = np.asarray(w, np.float32)
    Rr = _basis_feature_matrix()

    ones_row = np.ones((1, S), np.float32)
    in_maps = []
    for h in range(H):
        qT8 = np.ascontiguousarray(q[:, h, :].T) / np.float32(8.0)
        qaug = np.concatenate([qT8, ones_row], axis=0)
        kT = np.ascontiguousarray(k[:, h, :].T)
        uk8 = ((u[h] / np.float32(8.0)) @ kT).reshape(1, S)
        kaug = np.concatenate([kT, uk8], axis=0)
        vaug = np.concatenate([val[:, h, :], np.ones((S, 1), np.float32)],
                              axis=1).astype(ml_dtypes.bfloat16)
        w2r_qr = w[h] @ Rr.T
        vw8 = (v[h] @ w[h]) / np.float32(8.0)
        w2r_vr = (vw8 @ Rr.T).reshape(1, Q)
        w2r = np.concatenate([w2r_qr, w2r_vr],
                             axis=0).astype(ml_dtypes.bfloat16)
        in_maps.append({
            "qaug_f": np.ascontiguousarray(qaug),
            "qaug_b": np.ascontiguousarray(qaug).astype(ml_dtypes.bfloat16),
            "kaug": np.ascontiguousarray(kaug),
            "vaug": np.ascontiguousarray(vaug),
            "w2r": np.ascontiguousarray(w2r),
        })
    return in_maps


def kernel(query, key, value, u, v, w, _trace=False):
    nc = _build_nc()
    in_maps = _host_prep(query, key, value, u, v, w)
    res = run_bass_kernel_spmd(nc, in_maps, core_ids=list(range(H)),
                               trace=_trace)
    outs = np.stack([res.results[h]["out"] for h in range(H)])
    full = np.transpose(outs, (1, 0, 2))[None]
    out = np.ascontiguousarray(full.astype(np.float32))
    if _trace:
        return out, res
    return out


if __name__ == "__main__":
    rng = np.random.default_rng(0)
    ins = {
        "query": rng.standard_normal((1, S, H, D), np.float32),
        "key": rng.standard_normal((1, S, H, D), np.float32),
        "value": rng.standard_normal((1, S, H, D), np.float32),
        "u": rng.standard_normal((H, D), np.float32),
        "v": rng.standard_normal((H, D), np.float32),
        "w": rng.standard_normal((H, D, NB), np.float32),
    }
    out = kernel(**ins)
    print("out shape:", out.shape, "finite:", np.isfinite(out).all())
